# revision 5
# baseline (speedup 1.0000x reference)
"""Trainium2 Bass kernel for nn_BSLoss (Black-Scholes PINN loss on a 4096x4096 grid).

v2 strategy (8 NeuronCores, SPMD, S-sharded, bf16 transfers):
  - Host casts V to bf16: HBM traffic halves to ~4.2MB/core (DMA floor ~12us).
  - Each core: 512 rows (+1-row halos) x 4096 cols as 4 x 128-row tiles
    (step 126) + a 10-row strip. Per tile, two 2048-wide column groups.
  - residual/C_T = tri_S(V) + (V[:,j+1] - V[:,j-1]):
      * tri_S: one [128x128] bf16 stationary matmul per group (PE, PSUM f32).
      * t-shift: DVE tensor_tensor sub in bf16 (2x mode) into a w tile, then
        added into PSUM by a second matmul against an identity stationary
        (PE), except one strip group which uses a DVE STT add.
      * square+row-accumulate: ScalarE activation(Square, accum_out) per
        group; one group spilled to DVE copy+STT to balance engines.
  - Weight switches are minimized (tri x2 then I x2 per tile) and a few
    warmup matmuls run at t=0 to ramp the PE p-state before real data lands.
  - v DMAs stream on the Sync HWDGE queue in compute order (2 half-tiles per
    tile); weights + stats ride the Scalar queue.
  - Host applies row masks (x C_T^2) to the [128,10] per-row stats, sums in
    float64, and computes the O(N) boundary losses directly.

Grid boundary columns (t=0, t=4095) are excluded by construction; boundary
rows are masked out host-side.
"""
import os
import sys

if "/opt/trn_rl_repo" not in sys.path:
    sys.path.insert(0, "/opt/trn_rl_repo")

import numpy as np
import ml_dtypes

import concourse.mybir as mybir
import concourse.tile as tile
from concourse import bacc
from concourse.bass_utils import run_bass_kernel_spmd

BF16NP = ml_dtypes.bfloat16

# ---- problem constants (match the reference) ----
N_S, N_T = 4096, 4096
R, SIGMA, K, T_MAT, SMAX = 0.05, 0.2, 100.0, 1.0, 300.0
B_STR, ALPHA = K / SMAX, 0.5
L_PDE, L_BC, L_TC = 1.0, 10.0, 10.0
HUBER_DELTA = 0.01
SOFTPLUS_BETA = 50.0

N_CORES = 8
ROWS_PER_CORE = N_S // N_CORES          # 512
IN_ROWS = ROWS_PER_CORE + 2             # 514 (with halos)
P = 128
TILE_STARTS = [0, 126, 252, 378]        # full tiles; outputs local rows 1..504
STRIP_START = 504                       # strip tile rows 504..513 -> outputs 505..512
STRIP_K = 10
N_TILES = 5
C_T = (N_T - 1) / 2.0 / T_MAT           # 2047.5

# DMA column halves of each [*,4096] tile
H0_W = 2052                              # cols [0,2052); h1 = [2052,4096)
# column groups (global col base, width); cover interior cols 1..4094
G_BASE = [1, 2049]
G_W = [2048, 2046]
N_GROUPS = N_TILES * 2                   # stats columns: u = 2*tile + g
# engine assignment knobs
SQ_DVE = {(2, 1)}                        # (tile, g) squares spilled to DVE
STRIP_ADD_DVE_G = {1}                    # strip groups whose t-shift add is DVE STT
N_WARM = 4                               # PE warmup matmuls on the weights tile

F32 = mybir.dt.float32
BF16 = mybir.dt.bfloat16


def _solve_cubic(Q: float) -> float:
    c = -Q
    for _ in range(5):
        f = c ** 3 / 6.0 + c + Q
        df = 0.5 * c * c + 1.0
        c = c - f / df
    return c


C1 = _solve_cubic((B_STR - 0.0) / ALPHA)
C2 = _solve_cubic((B_STR - 1.0) / ALPHA)


def _stencil_coeffs(S: np.ndarray):
    """Per-row stencil coefficients / C_T (C_T folded out; re-applied via host mask)."""
    S = S.astype(np.float64)
    dS = 1.0 / (N_S - 1)
    L = C2 * S + C1 * (1.0 - S)
    dL = C2 - C1
    S_u = ALPHA * dL * (0.5 * L ** 2 + 1.0)
    S_uu = ALPHA * dL ** 2 * L
    e = 0.5 * SIGMA ** 2 * S ** 2
    f = R * S
    a_uu = e / S_u ** 2
    a_u = f / S_u - e * S_uu / S_u ** 3
    hi = a_uu / dS ** 2 + a_u / (2 * dS)
    lo = a_uu / dS ** 2 - a_u / (2 * dS)
    mid = -2.0 * a_uu / dS ** 2 - R
    return lo / C_T, mid / C_T, hi / C_T


_PROGRAM = None


def _patch_tail(tc_cls):
    """Cheaper kernel tail: drain + single barrier, no per-sem HW clears.
    Semaphore bookkeeping (free/poison) is kept so scheduling stays valid."""
    from concourse.vector_clock import ScopedClock as _SC

    def _drain_and_barrier(self, tick_clock, wait_clock):
        drain_inst = self.nc.sync.drain()
        wait_clock.add_sem_waits(drain_inst.ins, _SC({None: tick_clock.global_clock}))
        self.nc.all_engine_barrier()
        popped = self.nc._tile_sem_poison_stack.pop()
        assert popped is self._sem_poison
        sems = list(self.sems.allocated().values())
        sem_nums = [s.num if hasattr(s, "num") else s for s in sems]
        self.nc._state.prepend_free_semaphores(sem_nums)
        for poison_set in self.nc._tile_sem_poison_stack:
            poison_set.update(sem_nums)

    tc_cls._drain_and_barrier = _drain_and_barrier


def _build_program():
    if os.environ.get("BSLOSS_FAST_TAIL", "1") == "1":
        _patch_tail(tile.TileContext)
    nc = bacc.Bacc("TRN2", target_bir_lowering=False)

    v_in = nc.dram_tensor("v_in", [IN_ROWS, N_T], BF16, kind="ExternalInput")
    # 5 tridiag blocks + identity, all bf16
    w_in = nc.dram_tensor("w_in", [P, (N_TILES + 1) * P], BF16, kind="ExternalInput")
    stats_out = nc.dram_tensor("stats_out", [P, N_GROUPS], F32, kind="ExternalOutput")

    with tile.TileContext(nc) as tc:
        with (
            tc.tile_pool(name="vpool", bufs=1) as vpool,
            tc.tile_pool(name="wpool", bufs=1) as wpool,
            tc.tile_pool(name="scratch", bufs=2) as spool,
            tc.tile_pool(name="psum", bufs=2, space="PSUM") as psum_pool,
        ):
            wall = wpool.tile([P, (N_TILES + 1) * P], BF16)
            # weights ride the Scalar HWDGE queue (Sync streams the v tiles)
            nc.scalar.dma_start(wall[:], w_in[:])
            stats = wpool.tile([P, N_GROUPS], F32)

            # v tiles, DMA'd in compute order as column halves
            vh = {}
            for t in range(N_TILES):
                kdim = P if t < 4 else STRIP_K
                r0 = TILE_STARTS[t] if t < 4 else STRIP_START
                vt = vpool.tile([kdim, N_T], BF16, tag=f"v{t}")
                nc.sync.dma_start(vt[0:kdim, 0:H0_W],
                                  v_in[r0:r0 + kdim, 0:H0_W])
                nc.sync.dma_start(vt[0:kdim, H0_W:N_T],
                                  v_in[r0:r0 + kdim, H0_W:N_T])
                vh[t] = vt

            ident = wall[:, N_TILES * P:(N_TILES + 1) * P]

            # PE warmup: ramp the p-state before the first tile lands
            warm = psum_pool.tile([P, 2048], F32, tag="ps")
            for _ in range(N_WARM):
                nc.tensor.matmul(warm[:, 0:512], lhsT=ident,
                                 rhs=wall[:, 0:512], start=True, stop=True)

            for t in range(N_TILES):
                kdim = P if t < 4 else STRIP_K
                vt = vh[t]
                tri = wall[0:kdim, t * P:(t + 1) * P]

                # w[:, c] = V[:, c+1] - V[:, c-1]; two pieces so group 0 only
                # needs DMA half 0 (bf16 TT -> DVE 2x mode)
                wt = spool.tile([kdim, N_T], BF16, tag=f"w{t}")
                nc.vector.tensor_tensor(
                    out=wt[0:kdim, 1:H0_W - 1],
                    in0=vt[0:kdim, 2:H0_W],
                    in1=vt[0:kdim, 0:H0_W - 2],
                    op=mybir.AluOpType.subtract)
                nc.vector.tensor_tensor(
                    out=wt[0:kdim, H0_W - 1:N_T - 1],
                    in0=vt[0:kdim, H0_W:N_T],
                    in1=vt[0:kdim, H0_W - 2:N_T - 2],
                    op=mybir.AluOpType.subtract)

                ps0 = psum_pool.tile([P, 2048], F32, tag="ps")
                ps1 = psum_pool.tile([P, 2048], F32, tag="ps")
                ps = [ps0, ps1]
                strip_dve = {g for g in (0, 1)
                             if t == 4 and g in STRIP_ADD_DVE_G}

                def chunks(g):
                    # 512-wide matmul chunks (single PSUM bank each)
                    for off in range(0, G_W[g], 512):
                        yield off, min(512, G_W[g] - off)

                # tri matmuls for both groups (one weights load)
                for g in (0, 1):
                    for off, cw in chunks(g):
                        nc.tensor.matmul(
                            ps[g][:, off:off + cw], lhsT=tri,
                            rhs=vt[0:kdim,
                                   G_BASE[g] + off:G_BASE[g] + off + cw],
                            start=True, stop=(g in strip_dve))
                # t-shift adds (identity matmul, or DVE STT for strip g1)
                for g in (0, 1):
                    if g in strip_dve:
                        nc.vector.scalar_tensor_tensor(
                            out=ps[g][0:kdim, 0:G_W[g]],
                            in0=wt[0:kdim, G_BASE[g]:G_BASE[g] + G_W[g]],
                            scalar=1.0, in1=ps[g][0:kdim, 0:G_W[g]],
                            op0=mybir.AluOpType.mult, op1=mybir.AluOpType.add)
                    else:
                        for off, cw in chunks(g):
                            nc.tensor.matmul(
                                ps[g][:, off:off + cw],
                                lhsT=ident[0:kdim, :],
                                rhs=wt[0:kdim,
                                       G_BASE[g] + off:G_BASE[g] + off + cw],
                                start=False, stop=True)
                # square + per-row accumulate
                for g in (0, 1):
                    u = 2 * t + g
                    sq = spool.tile([P, 2048], F32, tag="sq")
                    if (t, g) in SQ_DVE:
                        rc = spool.tile([P, 2048], F32, tag="rc")
                        nc.vector.tensor_copy(rc[:, 0:G_W[g]], ps[g][:, 0:G_W[g]])
                        nc.vector.scalar_tensor_tensor(
                            out=sq[:, 0:G_W[g]], in0=rc[:, 0:G_W[g]], scalar=1.0,
                            in1=rc[:, 0:G_W[g]], op0=mybir.AluOpType.mult,
                            op1=mybir.AluOpType.mult,
                            accum_out=stats[:, u:u + 1])
                    else:
                        nc.scalar.activation(sq[:, 0:G_W[g]], ps[g][:, 0:G_W[g]],
                                             mybir.ActivationFunctionType.Square,
                                             accum_out=stats[:, u:u + 1])

            nc.scalar.dma_start(stats_out[:], stats[:])

    nc.compile()
    return nc


def _host_inputs_and_masks(V: np.ndarray, S: np.ndarray):
    lo, mid, hi = _stencil_coeffs(S)
    c2 = float(C_T) ** 2

    in_maps = []
    masks = []

    wtri = np.zeros((P, (N_TILES + 1) * P), np.float64)
    wtri[:, N_TILES * P:(N_TILES + 1) * P] = np.eye(P)

    for c in range(N_CORES):
        rows = np.clip(np.arange(512 * c - 1, 512 * c + 513), 0, N_S - 1)
        v_shard = np.ascontiguousarray(V[rows, :]).astype(BF16NP)

        w = wtri.copy()
        mask = np.zeros((P, N_GROUPS), np.float32)
        for t in range(N_TILES):
            if t < 4:
                t0, m_lo, m_hi = TILE_STARTS[t], 1, 126
            else:
                t0, m_lo, m_hi = STRIP_START, 1, 8
            for m in range(m_lo, m_hi + 1):
                L = t0 + m
                g = 512 * c - 1 + L
                if not (1 <= g <= N_S - 2):
                    continue
                w[m - 1, t * P + m] = lo[g]
                w[m, t * P + m] = mid[g]
                w[m + 1, t * P + m] = hi[g]
                mask[m, 2 * t:2 * t + 2] = c2
        in_maps.append({"v_in": v_shard,
                        "w_in": w.astype(np.float32).astype(BF16NP)})
        masks.append(mask)
    return in_maps, masks


_LAST_RESULTS = None  # stashed BassKernelResults (for the test harness)


def kernel(V_norm: np.ndarray, S_grid: np.ndarray, t_grid: np.ndarray):
    global _PROGRAM, _LAST_RESULTS

    V = np.asarray(V_norm, dtype=np.float32).reshape(N_S, N_T)
    S = np.asarray(S_grid, dtype=np.float32).reshape(N_S)
    t = np.asarray(t_grid, dtype=np.float32).reshape(N_T)

    if _PROGRAM is None:
        _PROGRAM = _build_program()
    nc = _PROGRAM

    in_maps, masks = _host_inputs_and_masks(V, S)
    trace = bool(os.environ.get("BSLOSS_TRACE"))
    res = run_bass_kernel_spmd(nc, in_maps, core_ids=list(range(N_CORES)),
                               trace=trace)
    _LAST_RESULTS = res

    pde_sum = 0.0
    for c in range(N_CORES):
        stats = res.results[c]["stats_out"].astype(np.float64)
        pde_sum += float((masks[c].astype(np.float64) * stats).sum())
    n_int = (N_S - 2) * (N_T - 2)
    pde_loss = pde_sum / n_int

    # ---- boundary losses on host (tiny O(N) edge terms), float64 ----
    V64 = V.astype(np.float64)
    S64 = S.astype(np.float64)
    t64 = t.astype(np.float64)

    loss_S0 = float((V64[0, :] ** 2).sum() / N_T)

    tau = 1.0 - t64
    V_ff = 1.0 - K * np.exp(-R * tau) / SMAX
    loss_Smax = float(((V64[N_S - 1, :] - V_ff) ** 2).sum() / N_T)

    x = SOFTPLUS_BETA * (S64 - K / SMAX)
    payoff = (np.maximum(x, 0.0) + np.log1p(np.exp(-np.abs(x)))) / SOFTPLUS_BETA
    diff_T = V64[:, N_T - 1] - payoff
    abs_d = np.abs(diff_T)
    huber = np.where(abs_d < HUBER_DELTA, 0.5 * diff_T ** 2,
                     HUBER_DELTA * (abs_d - 0.5 * HUBER_DELTA))
    loss_T = float(huber.sum() / N_S)

    total = L_PDE * pde_loss + L_BC * loss_Smax + L_TC * loss_T
    return (np.float32(total), np.float32(pde_loss), np.float32(loss_S0),
            np.float32(loss_Smax), np.float32(loss_T))
